# revision 1
# baseline (speedup 1.0000x reference)
"""CapsuleLinear (dynamic routing) Trainium2 kernel.

Reference computes priors = einsum('oli,bni->bonl', W, x) (302MB) then runs 3
routing iterations. We never materialize priors; per routing iteration:
    probs[n,o]   = softmax_o(logits[n,o])              (exp on ACT, Z on DVE)
    s[o,i]       = sum_n probs[n,o] * x[n,i]           (PE matmul, contract n)
    out[o,l]     = sum_i W[o,l,i] * s[o,i]             (DVE/GPSIMD mul+reduce)
    v            = squash(out)
    wv[o,i]      = sum_l W[o,l,i] * v[o,l]             (mul+reduce)
    logits[n,o] += sum_i x[n,i] * wv[o,i]              (PE matmul, contract i,
                                                        accumulates in PSUM)
Sharding: data-parallel over batch N=32 -> 4 batches per core on 8 cores.
Weight (64,32,32) replicated. No collectives.

Matmul operands are bf16 (measured end-to-end rel err ~5e-3; PSUM accumulation
stays fp32); the capsule-vector path (out-step, squash, wv) stays fp32.
sqrt(ns) is computed as exp(0.5*ln(ns)) so the whole kernel uses one ACT
table set (natural_log_exp_and_others) - no 1.3us table switches.

Per-core layouts:
  x_sb  [128(p), 4(b), 9(c), 32(i)]   x[b, c*128+p, i]          bf16
  xt_sb [32(i), 4(b), 9(c), 128(p)]   host-transposed x         bf16
  w_li  [128(b2*64+o), 32(l), 32(i)]  W pair-replicated         fp32
  w_il  [128(b2*64+o), 32(i), 32(l)]                            fp32
  logits PSUM [128(p), 4(b), 9(c), 64(o)] resident, fp32
  pair tiles [128(b2*64+o), 2(pair), ...] 2 batches stacked on partitions
"""

import os
import sys

for _p in ("/opt/trn_rl_repo",):
    if _p not in sys.path and os.path.isdir(_p):
        sys.path.insert(0, _p)

import numpy as np

import concourse.bacc as bacc
import concourse.bass as bass
import concourse.tile as tile
from concourse import mybir
from concourse.bass_utils import run_bass_kernel_spmd

CFG_BF16 = os.environ.get("K_BF16", "1") == "1"
CFG_TTR = os.environ.get("K_TTR", "0") == "1"  # TensorTensorReduce hangs TRN2 HW here
CFG_LNEXP = os.environ.get("K_LNEXP", "1") == "1"
CFG_GPS = os.environ.get("K_GPS", "1") == "1"
CFG_WBF = os.environ.get("K_WBF", "0") == "1"   # bf16 W / s / prod path (off: keeps rel err ~6e-3)
CFG_DBF = os.environ.get("K_DBF", "1") == "1"   # bf16 delta matmul (xt/wvT)
CFG_SQACC = os.environ.get("K_SQACC", "1") == "1"  # ns via ACT Square+accum

N_TOT, N_CAPS, I_LEN = 32, 1152, 32
O_CAPS, L_LEN = 64, 32
NCORES = 8
B = N_TOT // NCORES  # 4 batches per core
C = N_CAPS // 128    # 9 chunks of 128 input capsules
PAIRS = B // 2
FP = mybir.dt.float32
BF = mybir.dt.bfloat16
Exp = mybir.ActivationFunctionType.Exp
Ln = mybir.ActivationFunctionType.Ln
Square = mybir.ActivationFunctionType.Square
X = mybir.AxisListType.X
MUL = mybir.AluOpType.mult
BD = None  # set below: bf16 matmul-operand dtype, or fp32 when disabled


def build_nc():
    nc = bacc.Bacc("TRN2", target_bir_lowering=False, debug=True)
    BD = BF if CFG_BF16 else FP
    WD = BF if CFG_WBF else FP
    DD = BF if (CFG_BF16 and CFG_DBF) else FP
    x_nat_d = nc.dram_tensor("x_nat", [128, B, C, I_LEN], BD, kind="ExternalInput")
    xt_d = nc.dram_tensor("xt", [I_LEN, B, C, 128], DD, kind="ExternalInput")
    w_li_d = nc.dram_tensor("w_li", [128, L_LEN, I_LEN], WD, kind="ExternalInput")
    w_il_d = nc.dram_tensor("w_il", [128, I_LEN, L_LEN], WD, kind="ExternalInput")
    ident_d = nc.dram_tensor("ident", [128, 128], FP, kind="ExternalInput")
    out_d = nc.dram_tensor("out", [PAIRS, 128, L_LEN], FP, kind="ExternalOutput")

    with tile.TileContext(nc) as tc:
        with (
            tc.tile_pool(name="main", bufs=1) as pool,
            tc.tile_pool(name="psum", bufs=1, space="PSUM") as psum,
        ):
            x_sb = pool.tile([128, B, C, I_LEN], BD)
            xt_sb = pool.tile([I_LEN, B, C, 128], DD)
            wli_sb = pool.tile([128, L_LEN, I_LEN], WD)
            wil_sb = pool.tile([128, I_LEN, L_LEN], WD)
            ident = pool.tile([128, 128], FP)
            ones64 = pool.tile([128, O_CAPS], BD)
            shift = pool.tile([128, 1], FP)
            pexp = pool.tile([128, B, C, O_CAPS], BD)
            zsum = pool.tile([128, B, C], FP)
            rinv = pool.tile([128, B, C], FP)
            xr = pool.tile([128, B, C, I_LEN], BD)
            s_sb = pool.tile([128, PAIRS, I_LEN], WD)
            prod = pool.tile([128, PAIRS, L_LEN, I_LEN], WD)
            v_raw = pool.tile([128, PAIRS, L_LEN], FP)
            sq = pool.tile([128, PAIRS, L_LEN], FP)
            ns = pool.tile([128, PAIRS], FP)
            lnns = pool.tile([128, PAIRS], FP)
            vnorm = pool.tile([128, PAIRS], FP)
            denom = pool.tile([128, PAIRS], FP)
            rden = pool.tile([128, PAIRS], FP)
            factor = pool.tile([128, PAIRS], FP)
            v = pool.tile([128, PAIRS, L_LEN], FP)
            v_bf = pool.tile([128, PAIRS, L_LEN], WD)
            wprod = pool.tile([128, PAIRS, I_LEN, L_LEN], WD)
            wv = pool.tile([128, PAIRS, I_LEN], FP)
            wvt_sb = pool.tile([I_LEN, PAIRS, 128], DD)

            # logits PSUM, split into two 2-batch tiles so an iteration's
            # exp(b) only waits on its own half's delta matmuls. 18 chunks of
            # 256B per tile -> 2.25 banks (padded to 3). A matmul with
            # start=True lazily zeroes its whole bank, so emit start only on
            # the first chunk of each bank (r=0) and stop on the last.
            logits_ps = [
                psum.tile([128, 2, C, O_CAPS], FP, name=f"logits_ps{h}", tag=f"lg{h}")
                for h in range(2)
            ]
            # s (bytes 0..127) and wvT (bytes 512..1023) share a bank per pair;
            # the s -> v -> wv -> wvT dependency chain orders their lifetimes.
            u_ps = [
                psum.tile([128, 512], FP, name=f"u_ps{t}", tag=f"u_ps{t}")
                for t in range(PAIRS)
            ]
            s_ps = [u_ps[t][:, 0:I_LEN] for t in range(PAIRS)]
            wvt_ps = [u_ps[t][0:I_LEN, 128:256] for t in range(PAIRS)]

            dma = nc.sync
            # split/spread input DMAs across the three DMA-capable queues
            # (sync/scalar/gpsimd) in consumption order: x feeds the first
            # matmuls, w_li the out-step ~1us later, then w_il/ident/xt.
            # w_li/w_il are pair-replicated: fetch 64 rows from HBM, then a
            # local SBUF->SBUF DMA fills rows 64..127 (halves HBM traffic).
            nc.scalar.dma_start(out=wli_sb[0:64], in_=w_li_d[0:64])
            for b in range(B):
                dma.dma_start(out=x_sb[:, b], in_=x_nat_d[:, b])
            nc.scalar.dma_start(out=wil_sb[0:64], in_=w_il_d[0:64])
            nc.scalar.dma_start(out=wli_sb[64:128], in_=wli_sb[0:64])
            nc.scalar.dma_start(out=wil_sb[64:128], in_=wil_sb[0:64])
            nc.gpsimd.dma_start(out=ident[:], in_=ident_d[:])
            nc.gpsimd.dma_start(out=xt_sb[:], in_=xt_d[:])
            nc.vector.memset(ones64[:], 1.0)
            nc.vector.memset(shift[:], -40.0)

            for r in range(3):
                for b in range(B):
                    t, b2 = divmod(b, 2)
                    if r > 0:
                        # softmax numerator & partition function, per batch so
                        # the exp->Z->1/Z->xr->matmul chain pipelines over b.
                        # exp(l - 40): softmax-invariant shift keeps exp and
                        # 1/Z in fp32 range (logits span [-86, 92] here).
                        nc.scalar.activation(
                            out=pexp[:, b], in_=logits_ps[b // 2][:, b % 2],
                            func=Exp, bias=shift[:],
                        )
                        nc.vector.reduce_sum(out=zsum[:, b], in_=pexp[:, b], axis=X)
                        nc.vector.reciprocal(out=rinv[:, b], in_=zsum[:, b])
                        (nc.gpsimd if CFG_GPS else nc.vector).tensor_mul(
                            out=xr[:, b],
                            in0=x_sb[:, b],
                            in1=rinv[:, b].unsqueeze(-1).broadcast_to((128, C, I_LEN)),
                        )
                    # s[o,i] = sum_n probs * x  (iter 0: probs uniform -> ones)
                    for c in range(C):
                        nc.tensor.matmul(
                            out=s_ps[t][b2 * 64 : (b2 + 1) * 64, :],
                            lhsT=ones64[:] if r == 0 else pexp[:, b, c, :],
                            rhs=x_sb[:, b, c, :] if r == 0 else xr[:, b, c, :],
                            start=(c == 0),
                            stop=(c == C - 1),
                            tile_position=(0, 64 * b2),
                        )
                # PSUM -> SBUF (fold the uniform 1/64 prob into iter-0 copy)
                for t in range(PAIRS):
                    nc.scalar.mul(
                        out=s_sb[:, t, :],
                        in_=s_ps[t][:],
                        mul=(1.0 / 64 if r == 0 else 1.0),
                    )
                # out[o,l] = sum_i W[o,l,i] * s[o,i]; muls split DVE/GPSIMD
                for t in range(PAIRS):
                    nc.vector.tensor_mul(
                        out=prod[:, t],
                        in0=wli_sb[:],
                        in1=s_sb[:, t, :].unsqueeze(1).broadcast_to((128, L_LEN, I_LEN)),
                    )
                    nc.vector.reduce_sum(out=v_raw[:, t, :], in_=prod[:, t], axis=X)
                    # squash: factor = ||v||/(1+||v||^2); ns via fused TTR
                    if CFG_SQACC:
                        nc.scalar.activation(
                            out=sq[:, t],
                            in_=v_raw[:, t],
                            func=Square,
                            accum_out=ns[:, t : t + 1],
                        )
                    else:
                        nc.vector.tensor_mul(out=sq[:, t], in0=v_raw[:, t], in1=v_raw[:, t])
                        nc.vector.reduce_sum(out=ns[:, t : t + 1], in_=sq[:, t].unsqueeze(1), axis=X)
                # sqrt(ns) = exp(0.5*ln(ns)): stays in one ACT table set.
                # All squash ops split per pair so pair0's wv/delta chain
                # never waits on pair1's reduce.
                for t in range(PAIRS):
                    tsl = slice(t, t + 1)
                    if CFG_LNEXP:
                        nc.scalar.activation(out=lnns[:, tsl], in_=ns[:, tsl], func=Ln)
                        nc.scalar.activation(
                            out=vnorm[:, tsl], in_=lnns[:, tsl], func=Exp, scale=0.5
                        )
                    else:
                        nc.scalar.sqrt(out=vnorm[:, tsl], in_=ns[:, tsl])
                    nc.vector.tensor_scalar_add(
                        out=denom[:, tsl], in0=ns[:, tsl], scalar1=1.0
                    )
                    nc.vector.reciprocal(out=rden[:, tsl], in_=denom[:, tsl])
                    # v = (v_raw * ||v||) * (1/(1+||v||^2)) fused in one op
                    nc.vector.scalar_tensor_tensor(
                        out=(v[:, t] if r == 2 else v_bf[:, t]),
                        in0=v_raw[:, t],
                        scalar=vnorm[:, tsl],
                        in1=rden[:, tsl].broadcast_to((128, L_LEN)),
                        op0=MUL,
                        op1=MUL,
                    )
                if r == 2:
                    for t in range(PAIRS):
                        dma.dma_start(out=out_d[t], in_=v[:, t, :])
                else:
                    # wv[o,i] = sum_l W[o,l,i] * v[o,l]
                    for t in range(PAIRS):
                        nc.vector.tensor_mul(
                            out=wprod[:, t],
                            in0=wil_sb[:],
                            in1=v_bf[:, t, :]
                            .unsqueeze(1)
                            .broadcast_to((128, I_LEN, L_LEN)),
                        )
                        nc.vector.reduce_sum(out=wv[:, t, :], in_=wprod[:, t], axis=X)
                        nc.tensor.transpose(
                            out=wvt_ps[t][:], in_=wv[:, t, :], identity=ident[:]
                        )
                        nc.scalar.copy(out=wvt_sb[:, t, :], in_=wvt_ps[t][:])
                    # logits[n,o] += sum_i x[n,i] * wv[o,i]
                    # r0: one start/stop per 2KB psum bank (8 chunks per bank).
                    # r1: accumulate onto surviving has_written bits; the sim's
                    # group bookkeeping can't express re-opening, so skip it.
                    for b in range(B):
                        t, b2 = divmod(b, 2)
                        for c in range(C):
                            k = (b % 2) * C + c
                            nc.tensor.matmul(
                                out=logits_ps[b // 2][:, b % 2, c, :],
                                lhsT=xt_sb[:, b, c, :],
                                rhs=wvt_sb[:, t, b2 * 64 : (b2 + 1) * 64],
                                start=(r == 0 and k % 8 == 0),
                                stop=(r == 0 and (k % 8 == 7 or k == 2 * C - 1)),
                                skip_group_check=(r == 1),
                            )
    return nc


_NC = None


def get_nc():
    global _NC
    if _NC is None:
        _NC = build_nc()
    return _NC


def make_in_maps(x, weight):
    x = np.ascontiguousarray(x, dtype=np.float32)
    w = np.ascontiguousarray(weight, dtype=np.float32)
    w_li = np.tile(w.reshape(O_CAPS, L_LEN, I_LEN), (2, 1, 1))
    w_il = np.tile(w.transpose(0, 2, 1), (2, 1, 1))
    ident = np.eye(128, dtype=np.float32)
    in_maps = []
    for core in range(NCORES):
        xs = x[core * B : (core + 1) * B]  # [B, 1152, 32]
        xc = xs.reshape(B, C, 128, I_LEN)
        x_nat = np.ascontiguousarray(xc.transpose(2, 0, 1, 3))  # [128, B, C, 32]
        xt = np.ascontiguousarray(xc.transpose(3, 0, 1, 2))  # [32, B, C, 128]
        in_maps.append(
            {
                "x_nat": to_bf16(x_nat) if CFG_BF16 else x_nat,
                "xt": to_bf16(xt) if (CFG_BF16 and CFG_DBF) else xt,
                "w_li": to_bf16(w_li) if CFG_WBF else w_li,
                "w_il": to_bf16(w_il) if CFG_WBF else w_il,
                "ident": ident,
            }
        )
    return in_maps


def to_bf16(a):
    import ml_dtypes

    return a.astype(ml_dtypes.bfloat16)


def assemble(results):
    outs = []
    for core in range(NCORES):
        o = results[core]["out"]  # [PAIRS, 128, 32] -> [4, 64, 32]
        outs.append(np.asarray(o, dtype=np.float32).reshape(B, O_CAPS, L_LEN))
    return np.concatenate(outs, axis=0)


def _pin_act_table_set(nc):
    """Make Exp and Ln resolve to the one table set containing both
    (natural_log_exp_and_others), so the whole kernel runs on a single
    ACT table load instead of thrashing 1.3us loads between exp/ln sets.
    Mutates the cached dict in place; set indices stay aligned with
    act_info.json."""
    from concourse.hw_specs import get_activation_tables

    tabs = get_activation_tables(nc.m.arch)
    for name, funcs in tabs.items():
        if name != "natural_log_exp_and_others":
            funcs.discard(Exp)
            funcs.discard(Ln)
            funcs.discard(Square)
            funcs.discard(mybir.ActivationFunctionType.Copy)
            funcs.discard(mybir.ActivationFunctionType.Identity)


def run(x, weight, trace=False):
    nc = get_nc()
    if not nc.is_finalized():
        _pin_act_table_set(nc)
        nc.finalize()  # run Bacc lowering passes (wait splitting, reg alloc)
    res = run_bass_kernel_spmd(nc, make_in_maps(x, weight), list(range(NCORES)), trace=trace)
    return assemble(res.results), res


def kernel(x, weight):
    out, _ = run(x, weight)
    return out



# revision 6
# speedup vs baseline: 1.3223x; 1.3223x over previous
"""CapsuleLinear (dynamic routing) Trainium2 kernel.

Reference computes priors = einsum('oli,bni->bonl', W, x) (302MB) then runs 3
routing iterations. We never materialize priors; per routing iteration:
    probs[n,o]   = softmax_o(logits[n,o])              (exp on ACT, Z on DVE)
    s[o,i]       = sum_n probs[n,o] * x[n,i]           (PE matmul, contract n)
    out[o,l]     = sum_i W[o,l,i] * s[o,i]             (DVE mul+reduce)
    v            = squash(out)
    wv[o,i]      = sum_l W[o,l,i] * v[o,l]             (DVE mul+reduce)
    logits[n,o] += sum_i x[n,i] * wv[o,i]              (PE matmul, contract i,
                                                        accumulates in PSUM)
Sharding: data-parallel over batch N=32 -> 4 batches per core on 8 cores.
Weight (64,32,32) replicated. No collectives.

v2 (throughput rework, from trace analysis of the 62.7us baseline):
  - DVE was the bottleneck (42us busy): the out/wv mul+reduce ops ran fp32 at
    1 elem/cycle. All hot-path DVE tensors are now bf16 in AND out so the DVE
    2x_1p packed mode applies (all-2B packed operands; per the DVE microarch
    doc it covers tensor_tensor AND tensor_reduce). r=2's v_raw stays fp32
    (final output precision; reduce cost is dtype-independent in the worst
    case anyway).
  - ns = sum(v_raw^2) moved from ACT Square+READ_ACCUMULATOR (718ns/pair
    latency) to two tiny DVE ops.
  - exp/zsum/recip/xr run per PAIR (2 ops/iter instead of 4): same elements,
    half the fixed overheads.
  - w_li/w_il are host-replicated to 128 rows (the old SBUF->SBUF pair
    replication DMA landed at ~21us and gated the first out-step by ~3us).
  - x arrives as ONE 294KB DMA (4 per-batch DMAs serialized on one queue and
    the last landed at 17.3us; one transfer lands ~12us).
sqrt(ns) is computed as exp(0.5*ln(ns)) so the whole kernel uses one ACT
table set (natural_log_exp_and_others) - no 1.3us table switches.
"""

import os
import sys

for _p in ("/opt/trn_rl_repo",):
    if _p not in sys.path and os.path.isdir(_p):
        sys.path.insert(0, _p)

import numpy as np

import concourse.bacc as bacc
import concourse.bass as bass
import concourse.tile as tile
from concourse import mybir
from concourse.bass_utils import run_bass_kernel_spmd

# bf16 outputs on the big DVE reduces (enables 2x_1p if HW supports it on
# tensor_reduce; costs nothing if not). 0 = fp32 reduce outputs.
CFG_TRBF = os.environ.get("K_TRBF", "1") == "1"

N_TOT, N_CAPS, I_LEN = 32, 1152, 32
O_CAPS, L_LEN = 64, 32
NCORES = 8
B = N_TOT // NCORES  # 4 batches per core
C = N_CAPS // 128    # 9 chunks of 128 input capsules
PAIRS = B // 2       # 2 batches stacked on the 128 partitions
FP = mybir.dt.float32
BF = mybir.dt.bfloat16
Exp = mybir.ActivationFunctionType.Exp
Ln = mybir.ActivationFunctionType.Ln
X = mybir.AxisListType.X
MUL = mybir.AluOpType.mult


def build_nc():
    nc = bacc.Bacc("TRN2", target_bir_lowering=False, debug=True)
    RD = BF if CFG_TRBF else FP  # dtype of big reduce outputs (v_raw, wv, zsum)
    x_nat_d = nc.dram_tensor("x_nat", [128, PAIRS, 2, C, I_LEN], BF, kind="ExternalInput")
    xt_d = nc.dram_tensor("xt", [I_LEN, B, C, 128], BF, kind="ExternalInput")
    w_li_d = nc.dram_tensor("w_li", [128, L_LEN, I_LEN], BF, kind="ExternalInput")
    w_il_d = nc.dram_tensor("w_il", [128, I_LEN, L_LEN], BF, kind="ExternalInput")
    ident_d = nc.dram_tensor("ident", [128, 128], FP, kind="ExternalInput")
    out_d = nc.dram_tensor("out", [PAIRS, 128, L_LEN], FP, kind="ExternalOutput")

    with tile.TileContext(nc) as tc, nc.allow_low_precision(
        reason="bf16 hot path; end-to-end rel err budget 2e-2"
    ):
        with (
            tc.tile_pool(name="main", bufs=1) as pool,
            tc.tile_pool(name="psum", bufs=1, space="PSUM") as psum,
        ):
            x_sb = pool.tile([128, PAIRS, 2, C, I_LEN], BF)
            xt_sb = pool.tile([I_LEN, B, C, 128], BF)
            wli_sb = pool.tile([128, L_LEN, I_LEN], BF)
            wil_sb = pool.tile([128, I_LEN, L_LEN], BF)
            # PE transpose: out dtype must match identity, and fp32 rhs (wv)
            # requires fp32 identity -> the wv/wvT path stays fp32.
            ident = pool.tile([128, 128], FP)
            ones64 = pool.tile([128, O_CAPS], BF)
            shift = pool.tile([128, 1], FP)
            pexp = pool.tile([128, PAIRS, 2, C, O_CAPS], BF)
            zsum = pool.tile([128, PAIRS, 2, C], RD)
            rinv = pool.tile([128, PAIRS, 2, C], FP)
            xr = pool.tile([128, PAIRS, 2, C, I_LEN], BF)
            s_sb = pool.tile([128, PAIRS, I_LEN], BF)
            prod = pool.tile([128, PAIRS, L_LEN, I_LEN], BF)
            v_raw = pool.tile([128, PAIRS, L_LEN], RD)
            v_rawf = pool.tile([128, PAIRS, L_LEN], FP)
            sq = pool.tile([128, PAIRS, L_LEN], RD)
            sqf = pool.tile([128, PAIRS, L_LEN], FP)
            ns = pool.tile([128, PAIRS], FP)
            lnns = pool.tile([128, PAIRS], FP)
            vnorm = pool.tile([128, PAIRS], FP)
            denom = pool.tile([128, PAIRS], FP)
            rden = pool.tile([128, PAIRS], FP)
            v = pool.tile([128, PAIRS, L_LEN], FP)
            v_bf = pool.tile([128, PAIRS, L_LEN], BF)
            wprod = pool.tile([128, PAIRS, I_LEN, L_LEN], BF)
            wv = pool.tile([128, PAIRS, I_LEN], FP)
            wvt_sb = pool.tile([I_LEN, PAIRS, 128], BF)

            # logits PSUM, split into two 2-batch tiles so an iteration's
            # exp(pair) only waits on its own half's delta matmuls. A matmul
            # with start=True lazily zeroes its whole bank, so emit start only
            # on the first chunk of each bank (r=0) and stop on the last.
            logits_ps = [
                psum.tile([128, 2, C, O_CAPS], FP, name=f"logits_ps{h}", tag=f"lg{h}")
                for h in range(2)
            ]
            # s (bytes 0..127) and wvT (bytes 512..1023) share a bank per pair;
            # the s -> v -> wv -> wvT dependency chain orders their lifetimes.
            u_ps = [
                psum.tile([128, 512], FP, name=f"u_ps{t}", tag=f"u_ps{t}")
                for t in range(PAIRS)
            ]
            s_ps = [u_ps[t][:, 0:I_LEN] for t in range(PAIRS)]
            wvt_ps = [u_ps[t][0:I_LEN, 128:256] for t in range(PAIRS)]

            dma = nc.sync
            # x is the first thing compute needs: one big transfer, first on
            # its queue. Weights ride the scalar queue, xt/ident the gpsimd
            # queue (xt is only consumed ~6us in, by the delta matmuls).
            dma.dma_start(out=x_sb[:], in_=x_nat_d[:])
            nc.scalar.dma_start(out=wli_sb[:], in_=w_li_d[:])
            nc.scalar.dma_start(out=wil_sb[:], in_=w_il_d[:])
            nc.gpsimd.dma_start(out=xt_sb[:], in_=xt_d[:])
            nc.gpsimd.dma_start(out=ident[:], in_=ident_d[:])
            nc.vector.memset(ones64[:], 1.0)
            nc.vector.memset(shift[:], -40.0)

            for r in range(3):
                for h in range(PAIRS):
                    if r > 0:
                        # softmax numerator & partition function, per pair.
                        # exp(l - 40): softmax-invariant shift keeps exp and
                        # 1/Z in fp32 range (logits span [-86, 92] here).
                        nc.scalar.activation(
                            out=pexp[:, h], in_=logits_ps[h][:],
                            func=Exp, bias=shift[:],
                        )
                        nc.vector.reduce_sum(out=zsum[:, h], in_=pexp[:, h], axis=X)
                        nc.vector.reciprocal(out=rinv[:, h], in_=zsum[:, h])
                        nc.gpsimd.tensor_mul(
                            out=xr[:, h],
                            in0=x_sb[:, h],
                            in1=rinv[:, h].unsqueeze(-1).broadcast_to((128, 2, C, I_LEN)),
                        )
                    # s[o,i] = sum_n probs * x  (iter 0: probs uniform -> ones)
                    for b2 in range(2):
                        for c in range(C):
                            nc.tensor.matmul(
                                out=s_ps[h][b2 * 64 : (b2 + 1) * 64, :],
                                lhsT=ones64[:] if r == 0 else pexp[:, h, b2, c, :],
                                rhs=x_sb[:, h, b2, c, :] if r == 0 else xr[:, h, b2, c, :],
                                start=(c == 0),
                                stop=(c == C - 1),
                                tile_position=(0, 64 * b2),
                            )
                # PSUM -> SBUF (fold the uniform 1/64 prob into iter-0 copy)
                for t in range(PAIRS):
                    nc.scalar.mul(
                        out=s_sb[:, t, :],
                        in_=s_ps[t][:],
                        mul=(1.0 / 64 if r == 0 else 1.0),
                    )
                # out[o,l] = sum_i W[o,l,i] * s[o,i], all-bf16 on DVE
                for t in range(PAIRS):
                    nc.vector.tensor_mul(
                        out=prod[:, t],
                        in0=wli_sb[:],
                        in1=s_sb[:, t, :].unsqueeze(1).broadcast_to((128, L_LEN, I_LEN)),
                    )
                    if r == 2:
                        nc.vector.reduce_sum(out=v_rawf[:, t, :], in_=prod[:, t], axis=X)
                        nc.vector.tensor_mul(out=sqf[:, t], in0=v_rawf[:, t], in1=v_rawf[:, t])
                        nc.vector.reduce_sum(
                            out=ns[:, t : t + 1], in_=sqf[:, t].unsqueeze(1), axis=X
                        )
                    else:
                        nc.vector.reduce_sum(out=v_raw[:, t, :], in_=prod[:, t], axis=X)
                        nc.vector.tensor_mul(out=sq[:, t], in0=v_raw[:, t], in1=v_raw[:, t])
                        nc.vector.reduce_sum(
                            out=ns[:, t : t + 1], in_=sq[:, t].unsqueeze(1), axis=X
                        )
                # squash: factor = ||v||/(1+||v||^2); sqrt(ns) = exp(0.5*ln(ns))
                # stays in one ACT table set. Split per pair so pair0's
                # wv/delta chain never waits on pair1's reduce.
                for t in range(PAIRS):
                    tsl = slice(t, t + 1)
                    nc.scalar.activation(out=lnns[:, tsl], in_=ns[:, tsl], func=Ln)
                    nc.scalar.activation(
                        out=vnorm[:, tsl], in_=lnns[:, tsl], func=Exp, scale=0.5
                    )
                    nc.vector.tensor_scalar_add(
                        out=denom[:, tsl], in0=ns[:, tsl], scalar1=1.0
                    )
                    nc.vector.reciprocal(out=rden[:, tsl], in_=denom[:, tsl])
                    # v = (v_raw * ||v||) * (1/(1+||v||^2)) fused in one op
                    nc.vector.scalar_tensor_tensor(
                        out=(v[:, t] if r == 2 else v_bf[:, t]),
                        in0=(v_rawf[:, t] if r == 2 else v_raw[:, t]),
                        scalar=vnorm[:, tsl],
                        in1=rden[:, tsl].broadcast_to((128, L_LEN)),
                        op0=MUL,
                        op1=MUL,
                    )
                if r == 2:
                    for t in range(PAIRS):
                        dma.dma_start(out=out_d[t], in_=v[:, t, :])
                else:
                    # wv[o,i] = sum_l W[o,l,i] * v[o,l]
                    for t in range(PAIRS):
                        nc.vector.tensor_mul(
                            out=wprod[:, t],
                            in0=wil_sb[:],
                            in1=v_bf[:, t, :]
                            .unsqueeze(1)
                            .broadcast_to((128, I_LEN, L_LEN)),
                        )
                        nc.vector.reduce_sum(out=wv[:, t, :], in_=wprod[:, t], axis=X)
                        nc.tensor.transpose(
                            out=wvt_ps[t][:], in_=wv[:, t, :], identity=ident[:]
                        )
                        nc.scalar.copy(out=wvt_sb[:, t, :], in_=wvt_ps[t][:])
                    # logits[n,o] += sum_i x[n,i] * wv[o,i]
                    # r0: one start/stop per 2KB psum bank (8 chunks per bank).
                    # r1: accumulate onto surviving has_written bits; the sim's
                    # group bookkeeping can't express re-opening, so skip it.
                    for b in range(B):
                        t, b2 = divmod(b, 2)
                        for c in range(C):
                            k = b2 * C + c
                            nc.tensor.matmul(
                                out=logits_ps[t][:, b2, c, :],
                                lhsT=xt_sb[:, b, c, :],
                                rhs=wvt_sb[:, t, b2 * 64 : (b2 + 1) * 64],
                                start=(r == 0 and k % 8 == 0),
                                stop=(r == 0 and (k % 8 == 7 or k == 2 * C - 1)),
                                skip_group_check=(r == 1),
                            )
    return nc


_NC = None


def get_nc():
    global _NC
    if _NC is None:
        _NC = build_nc()
    return _NC


def to_bf16(a):
    import ml_dtypes

    return a.astype(ml_dtypes.bfloat16)


def make_in_maps(x, weight):
    x = np.ascontiguousarray(x, dtype=np.float32)
    w = np.ascontiguousarray(weight, dtype=np.float32)
    w_li = to_bf16(np.tile(w.reshape(O_CAPS, L_LEN, I_LEN), (2, 1, 1)))
    w_il = to_bf16(np.tile(w.transpose(0, 2, 1), (2, 1, 1)))
    ident = np.eye(128, dtype=np.float32)
    in_maps = []
    for core in range(NCORES):
        xs = x[core * B : (core + 1) * B]  # [B, 1152, 32]
        xc = xs.reshape(B, C, 128, I_LEN)
        x_nat = np.ascontiguousarray(xc.transpose(2, 0, 1, 3)).reshape(
            128, PAIRS, 2, C, I_LEN
        )
        xt = np.ascontiguousarray(xc.transpose(3, 0, 1, 2))  # [32, B, C, 128]
        in_maps.append(
            {
                "x_nat": to_bf16(x_nat),
                "xt": to_bf16(xt),
                "w_li": w_li,
                "w_il": w_il,
                "ident": ident,
            }
        )
    return in_maps


def assemble(results):
    outs = []
    for core in range(NCORES):
        o = results[core]["out"]  # [PAIRS, 128, 32] -> [4, 64, 32]
        outs.append(np.asarray(o, dtype=np.float32).reshape(B, O_CAPS, L_LEN))
    return np.concatenate(outs, axis=0)


def _pin_act_table_set(nc):
    """Make Exp and Ln resolve to the one table set containing both
    (natural_log_exp_and_others), so the whole kernel runs on a single
    ACT table load instead of thrashing 1.3us loads between exp/ln sets.
    Mutates the cached dict in place; set indices stay aligned with
    act_info.json."""
    from concourse.hw_specs import get_activation_tables

    tabs = get_activation_tables(nc.m.arch)
    for name, funcs in tabs.items():
        if name != "natural_log_exp_and_others":
            funcs.discard(Exp)
            funcs.discard(Ln)
            funcs.discard(mybir.ActivationFunctionType.Square)
            funcs.discard(mybir.ActivationFunctionType.Copy)
            funcs.discard(mybir.ActivationFunctionType.Identity)


def run(x, weight, trace=False):
    nc = get_nc()
    if not nc.is_finalized():
        _pin_act_table_set(nc)
        nc.finalize()  # run Bacc lowering passes (wait splitting, reg alloc)
    res = run_bass_kernel_spmd(nc, make_in_maps(x, weight), list(range(NCORES)), trace=trace)
    return assemble(res.results), res


def kernel(x, weight):
    out, _ = run(x, weight)
    return out


# revision 10
# speedup vs baseline: 1.4713x; 1.1127x over previous
"""CapsuleLinear (dynamic routing) Trainium2 kernel.

Reference computes priors = einsum('oli,bni->bonl', W, x) (302MB) then runs 3
routing iterations. We never materialize priors; per routing iteration:
    probs[n,o]   = softmax_o(logits[n,o])              (exp on ACT, Z on DVE)
    s[o,i]       = sum_n probs[n,o] * x[n,i]           (PE matmul, contract n)
    v_raw[o,l]   = sum_i W[o,l,i] * s[o,i]             (DVE mul+reduce)
    factor[o]    = ||v_raw||/(1+||v_raw||^2)           (squash, per-partition)
    wv[o,i]      = factor[o] * sum_l W[o,l,i]*v_raw[o,l]   (DVE mul+reduce,
                                                        factor pulled out of
                                                        the reduce: squash runs
                                                        CONCURRENT with the wv
                                                        mul+reduce)
    logits[n,o] += sum_i x[n,i] * wv[o,i]              (PE matmul, contract i,
                                                        accumulates in PSUM)
Sharding: data-parallel over batch N=32 -> 4 batches per core on 8 cores.
Weight (64,32,32) replicated. No collectives.

v3 (latency rework; v2 was dependency-bound at 51% DVE utilization):
  - squash factor applied AFTER the wv reduce (it is per-o, i.e. a per-
    partition scalar): the sq/ns/ln/exp/recip chain overlaps the wv
    mul+reduce instead of preceding it. v itself is only materialized at r=2.
  - exp/zsum/recip/xr at per-BATCH granularity, and each batch's exp for the
    NEXT iteration is emitted right after that batch's delta matmuls
    (software pipelining across iterations - engine queues are in-order).
  - s PSUM->SBUF copies moved to the gpsimd queue (ACT was congested at the
    iteration boundary); the uniform 1/64 prob of iter 0 is folded into the
    ones lhsT (memset 1/64) so the copy is a plain copy every iteration.
  - x arrives as two DMAs on two queues (sync + vector).
All big DVE ops are bf16-in/bf16-out (2x packed mode, ~0.52ns/elem); reduces
are 1x at any dtype (measured) so v_raw/zsum dtype only matters downstream.
sqrt(ns) is computed as exp(0.5*ln(ns)) so the whole kernel uses one ACT
table set (natural_log_exp_and_others) - no 1.3us table switches.
"""

import os
import sys

for _p in ("/opt/trn_rl_repo",):
    if _p not in sys.path and os.path.isdir(_p):
        sys.path.insert(0, _p)

import numpy as np

import concourse.bacc as bacc
import concourse.bass as bass
import concourse.tile as tile
from concourse import mybir
from concourse.bass_utils import run_bass_kernel_spmd

N_TOT, N_CAPS, I_LEN = 32, 1152, 32
O_CAPS, L_LEN = 64, 32
NCORES = 8
B = N_TOT // NCORES  # 4 batches per core
C = N_CAPS // 128    # 9 chunks of 128 input capsules
PAIRS = B // 2       # 2 batches stacked on the 128 partitions
FP = mybir.dt.float32
BF = mybir.dt.bfloat16
Exp = mybir.ActivationFunctionType.Exp
Ln = mybir.ActivationFunctionType.Ln
X = mybir.AxisListType.X
MUL = mybir.AluOpType.mult


def build_nc():
    nc = bacc.Bacc("TRN2", target_bir_lowering=False, debug=True)
    x_nat_d = nc.dram_tensor("x_nat", [128, PAIRS, 2, C, I_LEN], BF, kind="ExternalInput")
    xt_d = nc.dram_tensor("xt", [I_LEN, B, C, 128], BF, kind="ExternalInput")
    w_li_d = nc.dram_tensor("w_li", [128, L_LEN, I_LEN], BF, kind="ExternalInput")
    w_il_d = nc.dram_tensor("w_il", [128, I_LEN, L_LEN], BF, kind="ExternalInput")
    ident_d = nc.dram_tensor("ident", [128, 128], FP, kind="ExternalInput")
    out_d = nc.dram_tensor("out", [PAIRS, 128, L_LEN], FP, kind="ExternalOutput")

    with tile.TileContext(nc) as tc, nc.allow_low_precision(
        reason="bf16 hot path; end-to-end rel err budget 2e-2"
    ):
        with (
            tc.tile_pool(name="main", bufs=1) as pool,
            tc.tile_pool(name="psum", bufs=1, space="PSUM") as psum,
        ):
            x_sb = pool.tile([128, PAIRS, 2, C, I_LEN], BF)
            xt_sb = pool.tile([I_LEN, B, C, 128], BF)
            wli_sb = pool.tile([128, L_LEN, I_LEN], BF)
            wil_sb = pool.tile([128, I_LEN, L_LEN], BF)
            # PE transpose: out dtype must match identity, and fp32 rhs (wv)
            # requires fp32 identity -> the wv/wvT path stays fp32.
            ident = pool.tile([128, 128], FP)
            ones64 = pool.tile([128, O_CAPS], BF)
            shift = pool.tile([128, 1], FP)
            pexp = pool.tile([128, PAIRS, 2, C, O_CAPS], BF)
            zsum = pool.tile([128, PAIRS, 2, C], BF)
            rinv = pool.tile([128, PAIRS, 2, C], FP)
            xr = pool.tile([128, PAIRS, 2, C, I_LEN], BF)
            s_sb = pool.tile([128, PAIRS, I_LEN], BF)
            prod = pool.tile([128, PAIRS, L_LEN, I_LEN], BF)
            v_raw = pool.tile([128, PAIRS, L_LEN], BF)
            v_rawf = pool.tile([128, PAIRS, L_LEN], FP)
            sq = pool.tile([128, PAIRS, L_LEN], BF)
            sqf = pool.tile([128, PAIRS, L_LEN], FP)
            ns = pool.tile([128, PAIRS], FP)
            lnns = pool.tile([128, PAIRS], FP)
            vnorm = pool.tile([128, PAIRS], FP)
            denom = pool.tile([128, PAIRS], FP)
            rden = pool.tile([128, PAIRS], FP)
            v = pool.tile([128, PAIRS, L_LEN], FP)
            wprod = pool.tile([128, PAIRS, I_LEN, L_LEN], BF)
            wvraw = pool.tile([128, PAIRS, I_LEN], FP)
            wv = pool.tile([128, PAIRS, I_LEN], FP)
            wvt_sb = pool.tile([I_LEN, PAIRS, 128], BF)

            # logits PSUM, split into two 2-batch tiles so an iteration's
            # exp(b) only waits on its own half's delta matmuls. A matmul
            # with start=True lazily zeroes its whole bank, so emit start only
            # on the first chunk of each bank (r=0) and stop on the last.
            logits_ps = [
                psum.tile([128, 2, C, O_CAPS], FP, name=f"logits_ps{h}", tag=f"lg{h}")
                for h in range(2)
            ]
            # s (bytes 0..127) and wvT (bytes 512..1023) share a bank per pair;
            # the s -> v_raw -> wv -> wvT dependency chain orders their
            # lifetimes.
            u_ps = [
                psum.tile([128, 512], FP, name=f"u_ps{t}", tag=f"u_ps{t}")
                for t in range(PAIRS)
            ]
            s_ps = [u_ps[t][:, 0:I_LEN] for t in range(PAIRS)]
            wvt_ps = [u_ps[t][0:I_LEN, 128:256] for t in range(PAIRS)]

            dma = nc.sync
            # x is the first thing compute needs: two transfers on two queues,
            # first in each. Weights ride the scalar queue, xt/ident the
            # gpsimd queue (xt is only consumed ~6us in by the delta matmuls).
            dma.dma_start(out=x_sb[:, 0], in_=x_nat_d[:, 0])
            nc.gpsimd.dma_start(out=x_sb[:, 1], in_=x_nat_d[:, 1])
            nc.scalar.dma_start(out=wli_sb[:], in_=w_li_d[:])
            nc.scalar.dma_start(out=wil_sb[:], in_=w_il_d[:])
            nc.gpsimd.dma_start(out=xt_sb[:], in_=xt_d[:])
            nc.gpsimd.dma_start(out=ident[:], in_=ident_d[:])
            # iter-0 probs are uniform 1/64: fold into the ones lhsT so the
            # s PSUM->SBUF copy is a plain copy every iteration.
            nc.vector.memset(ones64[:], 1.0 / 64)
            nc.vector.memset(shift[:], -40.0)

            for r in range(3):
                # --- front: probs -> s matmuls (pexp was written in r-1's
                # tail; engine queues are in-order so per-batch granularity
                # keeps the refill chain short).
                for b in range(B):
                    t, b2 = divmod(b, 2)
                    if r > 0:
                        nc.vector.reduce_sum(
                            out=zsum[:, t, b2], in_=pexp[:, t, b2], axis=X
                        )
                        nc.vector.reciprocal(out=rinv[:, t, b2], in_=zsum[:, t, b2])
                        nc.gpsimd.tensor_mul(
                            out=xr[:, t, b2],
                            in0=x_sb[:, t, b2],
                            in1=rinv[:, t, b2]
                            .unsqueeze(-1)
                            .broadcast_to((128, C, I_LEN)),
                        )
                    for c in range(C):
                        nc.tensor.matmul(
                            out=s_ps[t][b2 * 64 : (b2 + 1) * 64, :],
                            lhsT=ones64[:] if r == 0 else pexp[:, t, b2, c, :],
                            rhs=x_sb[:, t, b2, c, :] if r == 0 else xr[:, t, b2, c, :],
                            start=(c == 0),
                            stop=(c == C - 1),
                            tile_position=(0, 64 * b2),
                        )
                # s PSUM->SBUF (+ fp32->bf16) copies: GPSIMD can't read PSUM,
                # so these ride ACT; both emitted up front so pair1's copy
                # isn't queued behind pair0's tail ACT work.
                for t in range(PAIRS):
                    nc.scalar.copy(out=s_sb[:, t, :], in_=s_ps[t][:])
                # --- per pair: out-step, squash (concurrent with wv-step),
                # wv-step, agreement matmuls, next iteration's exp.
                for t in range(PAIRS):
                    tsl = slice(t, t + 1)
                    nc.vector.tensor_mul(
                        out=prod[:, t],
                        in0=wli_sb[:],
                        in1=s_sb[:, t, :].unsqueeze(1).broadcast_to((128, L_LEN, I_LEN)),
                    )
                    vr_out = v_rawf if r == 2 else v_raw
                    nc.vector.reduce_sum(out=vr_out[:, t, :], in_=prod[:, t], axis=X)
                    # squash pieces: ns = sum(v_raw^2); vnorm = exp(0.5*ln(ns));
                    # rden = 1/(1+ns). Emitted before the wv mul+reduce on DVE
                    # so ACT's ln/exp overlap the wv ops.
                    sq_out = sqf if r == 2 else sq
                    nc.vector.tensor_mul(
                        out=sq_out[:, t], in0=vr_out[:, t], in1=vr_out[:, t]
                    )
                    nc.vector.reduce_sum(
                        out=ns[:, tsl], in_=sq_out[:, t].unsqueeze(1), axis=X
                    )
                    nc.vector.tensor_scalar_add(
                        out=denom[:, tsl], in0=ns[:, tsl], scalar1=1.0
                    )
                    nc.vector.reciprocal(out=rden[:, tsl], in_=denom[:, tsl])
                    nc.scalar.activation(out=lnns[:, tsl], in_=ns[:, tsl], func=Ln)
                    nc.scalar.activation(
                        out=vnorm[:, tsl], in_=lnns[:, tsl], func=Exp, scale=0.5
                    )
                    if r == 2:
                        # v = (v_raw * ||v||) * 1/(1+||v||^2)
                        nc.vector.tensor_scalar(
                            out=v[:, t],
                            in0=v_rawf[:, t],
                            scalar1=vnorm[:, tsl],
                            scalar2=rden[:, tsl],
                            op0=MUL,
                            op1=MUL,
                        )
                        dma.dma_start(out=out_d[t], in_=v[:, t, :])
                        continue
                    # wv-step on the unsquashed v_raw; factor folded in after.
                    nc.vector.tensor_mul(
                        out=wprod[:, t],
                        in0=wil_sb[:],
                        in1=v_raw[:, t, :].unsqueeze(1).broadcast_to((128, I_LEN, L_LEN)),
                    )
                    nc.vector.reduce_sum(out=wvraw[:, t, :], in_=wprod[:, t], axis=X)
                    nc.vector.tensor_scalar(
                        out=wv[:, t, :],
                        in0=wvraw[:, t, :],
                        scalar1=vnorm[:, tsl],
                        scalar2=rden[:, tsl],
                        op0=MUL,
                        op1=MUL,
                    )
                    nc.tensor.transpose(
                        out=wvt_ps[t][:], in_=wv[:, t, :], identity=ident[:]
                    )
                    nc.scalar.copy(out=wvt_sb[:, t, :], in_=wvt_ps[t][:])
                    # logits[n,o] += sum_i x[n,i] * wv[o,i], then this batch's
                    # exp for the NEXT iteration immediately behind its chunks.
                    # r0: one start/stop per 2KB psum bank (8 chunks per bank).
                    # r1: accumulate onto surviving has_written bits; the sim's
                    # group bookkeeping can't express re-opening, so skip it.
                    # (PSUM banks span the two b2 halves, so the exps can only
                    # go after all 18 chunks close their accumulation groups.)
                    for b2 in range(2):
                        for c in range(C):
                            k = b2 * C + c
                            nc.tensor.matmul(
                                out=logits_ps[t][:, b2, c, :],
                                lhsT=xt_sb[:, 2 * t + b2, c, :],
                                rhs=wvt_sb[:, t, b2 * 64 : (b2 + 1) * 64],
                                start=(r == 0 and k % 8 == 0),
                                stop=(r == 0 and (k % 8 == 7 or k == 2 * C - 1)),
                                skip_group_check=(r == 1),
                            )
                    for b2 in range(2):
                        nc.scalar.activation(
                            out=pexp[:, t, b2],
                            in_=logits_ps[t][:, b2],
                            func=Exp,
                            bias=shift[:],
                        )
    return nc


_NC = None


def get_nc():
    global _NC
    if _NC is None:
        _NC = build_nc()
    return _NC


def to_bf16(a):
    import ml_dtypes

    return a.astype(ml_dtypes.bfloat16)


def make_in_maps(x, weight):
    x = np.ascontiguousarray(x, dtype=np.float32)
    w = np.ascontiguousarray(weight, dtype=np.float32)
    w_li = to_bf16(np.tile(w.reshape(O_CAPS, L_LEN, I_LEN), (2, 1, 1)))
    w_il = to_bf16(np.tile(w.transpose(0, 2, 1), (2, 1, 1)))
    ident = np.eye(128, dtype=np.float32)
    in_maps = []
    for core in range(NCORES):
        xs = x[core * B : (core + 1) * B]  # [B, 1152, 32]
        xc = xs.reshape(B, C, 128, I_LEN)
        x_nat = np.ascontiguousarray(xc.transpose(2, 0, 1, 3)).reshape(
            128, PAIRS, 2, C, I_LEN
        )
        xt = np.ascontiguousarray(xc.transpose(3, 0, 1, 2))  # [32, B, C, 128]
        in_maps.append(
            {
                "x_nat": to_bf16(x_nat),
                "xt": to_bf16(xt),
                "w_li": w_li,
                "w_il": w_il,
                "ident": ident,
            }
        )
    return in_maps


def assemble(results):
    outs = []
    for core in range(NCORES):
        o = results[core]["out"]  # [PAIRS, 128, 32] -> [4, 64, 32]
        outs.append(np.asarray(o, dtype=np.float32).reshape(B, O_CAPS, L_LEN))
    return np.concatenate(outs, axis=0)


def _pin_act_table_set(nc):
    """Make Exp and Ln resolve to the one table set containing both
    (natural_log_exp_and_others), so the whole kernel runs on a single
    ACT table load instead of thrashing 1.3us loads between exp/ln sets.
    Mutates the cached dict in place; set indices stay aligned with
    act_info.json."""
    from concourse.hw_specs import get_activation_tables

    tabs = get_activation_tables(nc.m.arch)
    for name, funcs in tabs.items():
        if name != "natural_log_exp_and_others":
            funcs.discard(Exp)
            funcs.discard(Ln)
            funcs.discard(mybir.ActivationFunctionType.Square)
            funcs.discard(mybir.ActivationFunctionType.Copy)
            funcs.discard(mybir.ActivationFunctionType.Identity)


def run(x, weight, trace=False):
    nc = get_nc()
    if not nc.is_finalized():
        _pin_act_table_set(nc)
        nc.finalize()  # run Bacc lowering passes (wait splitting, reg alloc)
    res = run_bass_kernel_spmd(nc, make_in_maps(x, weight), list(range(NCORES)), trace=trace)
    return assemble(res.results), res


def kernel(x, weight):
    out, _ = run(x, weight)
    return out
